# revision 4
# baseline (speedup 1.0000x reference)
"""Multi-head attention (B=2, S=2048, H=1024, 16 heads) on 8 TRN2 NeuronCores.

Sharding: data parallel on batch (2) x tensor parallel on heads (4 heads/core,
Megatron column-split qkv, row-split wo). Host pre-transposes x/y, pre-scales
wq by dh^-0.5, and sum-reduces the 4 partial outputs per batch element.

Per-core kernel (all matmuls in fp32r, 1 cycle/row):
  Phase A: QT/KT in [head-pair-dims(128) x S] transposed layout, V in natural
           [S x dims] layout with a fused ones column (scaled by exp(bias) so
           the additive attention bias is exact).
  Phase B: per 512-wide q-block and head pair: row-tiled (2-head packed)
           QK^T -> logitsT psum [128,1024] -> one ACT exp per pair ->
           PV matmul with fused denominator row -> gpsimd partition_broadcast
           normalize -> pair-stacked output projection.
"""
import sys
sys.path.insert(0, '/opt/trn_rl_repo')
from contextlib import ExitStack

import numpy as np

import concourse.bacc as bacc
import concourse.tile as tile
from concourse import mybir
from concourse import bass_utils

B, S, H, NH = 2, 2048, 1024, 16
DH = H // NH            # 64
NCORES = 8
HPC = NH // (NCORES // B)   # 4 heads per core
C = HPC * DH            # 256 projected cols per core
KT_H = H // 128         # 8 contraction tiles over H
SK = S // 128           # 16 s-subtiles
JBLK = 512
NJ = S // JBLK          # 4 q-blocks
F32 = mybir.dt.float32
F32R = mybir.dt.float32r

_CACHE = {}


def _build():
    nc = bacc.Bacc('TRN2', debug=False, num_devices=NCORES)
    xT = nc.dram_tensor('xT', [H, S], F32R, kind='ExternalInput')
    yT = nc.dram_tensor('yT', [H, S], F32R, kind='ExternalInput')
    wq = nc.dram_tensor('wq', [H, C], F32R, kind='ExternalInput')
    wk = nc.dram_tensor('wk', [H, C], F32R, kind='ExternalInput')
    wv = nc.dram_tensor('wv', [H, C], F32R, kind='ExternalInput')
    wo = nc.dram_tensor('wo', [C, H], F32R, kind='ExternalInput')
    ebias = nc.dram_tensor('ebias', [128, SK], F32, kind='ExternalInput')
    out = nc.dram_tensor('out', [S, H], F32, kind='ExternalOutput')

    with tile.TileContext(nc) as tc, ExitStack() as ctx:
        res = ctx.enter_context(tc.tile_pool(name='res', bufs=1))
        stream = ctx.enter_context(tc.tile_pool(name='stream', bufs=3))
        expool = ctx.enter_context(tc.tile_pool(name='expool', bufs=3))
        ctxpool = ctx.enter_context(tc.tile_pool(name='ctxpool', bufs=2))
        small = ctx.enter_context(tc.tile_pool(name='small', bufs=2))
        outpool = ctx.enter_context(tc.tile_pool(name='outpool', bufs=3))

        # ---- resident weights ----
        wq_r = res.tile([128, KT_H, C], F32R, tag='wq')
        wk_r = res.tile([128, KT_H, C], F32R, tag='wk')
        wv_r = res.tile([128, KT_H, C], F32R, tag='wv')
        nc.sync.dma_start(out=wq_r, in_=wq.ap().rearrange('(t p) c -> p t c', p=128))
        nc.sync.dma_start(out=wk_r, in_=wk.ap().rearrange('(t p) c -> p t c', p=128))
        nc.sync.dma_start(out=wv_r, in_=wv.ap().rearrange('(t p) c -> p t c', p=128))
        wo_r = res.tile([128, 2, H], F32R, tag='wo')
        nc.sync.dma_start(out=wo_r, in_=wo.ap().rearrange('(t p) n -> p t n', p=128))
        eb = res.tile([128, SK], F32, tag='eb')
        nc.sync.dma_start(out=eb, in_=ebias.ap())
        ones4 = res.tile([128, HPC, 1], F32, tag='ones4')
        nc.vector.memset(ones4, 1.0)

        # ---- resident activations ----
        QT = [res.tile([128, S], F32R, tag=f'qt{p}', name=f'qt{p}') for p in range(2)]
        KTs = [res.tile([128, S], F32R, tag=f'kt{p}', name=f'kt{p}') for p in range(2)]
        # V tiles: [s-subtile 128, 4 heads x (64 v-dims + 1 e^bias col)]
        v_sb = [res.tile([128, HPC, DH + 1], F32R, tag=f'v{i}', name=f'v{i}') for i in range(SK)]

        xT_ap, yT_ap = xT.ap(), yT.ap()

        # ---- Phase A: projections ----
        actx = ExitStack()
        ps_proj = actx.enter_context(tc.tile_pool(name='ps_proj', bufs=1, space='PSUM'))
        for j4 in range(NJ):
            js = slice(j4 * JBLK, (j4 + 1) * JBLK)
            psq = [ps_proj.tile([128, JBLK], F32, tag='psq', bufs=2, name=f'psq{j4}_{i}') for i in range(2)]
            psk = [ps_proj.tile([128, JBLK], F32, tag='psk', bufs=2, name=f'psk{j4}_{i}') for i in range(2)]
            psv = [ps_proj.tile([128, C], F32, tag='psv', bufs=4, name=f'psv{j4}_{i}') for i in range(4)]
            for k in range(KT_H):
                ks = slice(k * 128, (k + 1) * 128)
                xt = stream.tile([128, JBLK], F32R, tag='xt')
                yt = stream.tile([128, JBLK], F32R, tag='yt')
                nc.sync.dma_start(out=xt, in_=xT_ap[ks, js])
                nc.sync.dma_start(out=yt, in_=yT_ap[ks, js])
                for p in range(2):
                    cs = slice(p * 128, (p + 1) * 128)
                    nc.tensor.matmul(psq[p], wq_r[:, k, cs], xt,
                                     start=(k == 0), stop=(k == KT_H - 1))
                    nc.tensor.matmul(psk[p], wk_r[:, k, cs], yt,
                                     start=(k == 0), stop=(k == KT_H - 1))
                for m in range(4):
                    nc.tensor.matmul(psv[m], yt[:, m * 128:(m + 1) * 128], wv_r[:, k, :],
                                     start=(k == 0), stop=(k == KT_H - 1))
            for p in range(2):
                nc.vector.tensor_copy(QT[p][:, js], psq[p])
                nc.vector.tensor_copy(KTs[p][:, js], psk[p])
            for m in range(4):
                sub = j4 * 4 + m
                nc.vector.tensor_scalar_mul(
                    v_sb[sub][:, :, 0:DH],
                    psv[m].rearrange('p (h c) -> p h c', h=HPC),
                    eb[:, sub:sub + 1])
                nc.vector.tensor_scalar_mul(v_sb[sub][:, :, DH:DH + 1], ones4,
                                            eb[:, sub:sub + 1])

        actx.close()

        # ---- Phase B: attention + output projection ----
        ps_qk = ctx.enter_context(tc.tile_pool(name='ps_qk', bufs=2, space='PSUM'))
        ps_pv = ctx.enter_context(tc.tile_pool(name='ps_pv', bufs=4, space='PSUM'))
        pend_out = None  # deferred out-proj emission for previous J

        def emit_out(J, ctx_tiles):
            for m in range(4):
                ms = slice(m * 128, (m + 1) * 128)
                for n in range(2):
                    ns = slice(n * JBLK, (n + 1) * JBLK)
                    pso = ps_pv.tile([128, JBLK], F32, tag='pv')
                    for p in range(2):
                        nc.tensor.matmul(pso, ctx_tiles[p][:, ms], wo_r[:, p, ns],
                                         start=(p == 0), stop=(p == 1))
                    ob = outpool.tile([128, JBLK], F32, tag='ob')
                    nc.vector.tensor_copy(ob, pso)
                    nc.sync.dma_start(out=out.ap()[J * JBLK + m * 128:
                                                   J * JBLK + (m + 1) * 128, ns],
                                      in_=ob)

        for J in range(NJ):
            js = slice(J * JBLK, (J + 1) * JBLK)
            ctx_tiles = []
            for p in range(2):
                pv0 = ps_pv.tile([128, JBLK], F32, tag='pv')
                pv1 = ps_pv.tile([128, JBLK], F32, tag='pv')
                for kk in range(SK):
                    kks = slice(kk * 128, (kk + 1) * 128)
                    psl = ps_qk.tile([128, 2 * JBLK], F32, tag='qk')
                    nc.tensor.matmul(psl[:, 0:JBLK],
                                     KTs[p][0:64, kks], QT[p][0:64, js],
                                     start=True, stop=True, tile_position=(0, 0))
                    nc.tensor.matmul(psl[:, JBLK:2 * JBLK],
                                     KTs[p][64:128, kks], QT[p][64:128, js],
                                     start=True, stop=True, tile_position=(64, 0))
                    ex = expool.tile([128, 2 * JBLK], F32R, tag='ex')
                    nc.scalar.activation(ex, psl, mybir.ActivationFunctionType.Exp)
                    for hh, pv in enumerate((pv0, pv1)):
                        hcol = 2 * p + hh
                        nc.tensor.matmul(
                            pv[0:DH + 1, :],
                            v_sb[kk][:, hcol, :],
                            ex[:, hh * JBLK:(hh + 1) * JBLK],
                            start=(kk == 0), stop=(kk == SK - 1))
                # normalize: ctxT[d, q] * (1/denom[q]) via partition broadcast
                ct = ctxpool.tile([128, JBLK], F32R, tag=f'ctx{p}')
                for hh, pv in enumerate((pv0, pv1)):
                    rec = small.tile([128, JBLK], F32, tag='rec')
                    nc.vector.reciprocal(rec[DH:DH + 1, :], pv[DH:DH + 1, :])
                    bcs = small.tile([128, JBLK], F32, tag='bcs')
                    nc.sync.dma_start(out=bcs[0:1, :], in_=rec[DH:DH + 1, :])
                    bc = small.tile([128, JBLK], F32, tag='bc')
                    nc.gpsimd.partition_broadcast(bc[0:DH, :], bcs[0:1, :])
                    if hh == 0:
                        nc.vector.tensor_mul(ct[0:DH, :], pv[0:DH, :], bc[0:DH, :])
                    else:
                        tmp = small.tile([128, JBLK], F32R, tag='tmp')
                        nc.vector.tensor_mul(tmp[0:DH, :], pv[0:DH, :], bc[0:DH, :])
                        nc.sync.dma_start(out=ct[DH:128, :], in_=tmp[0:DH, :])
                ctx_tiles.append(ct)
                if p == 0 and pend_out is not None:
                    emit_out(*pend_out)
                    pend_out = None
            pend_out = (J, ctx_tiles)
        emit_out(*pend_out)

    nc.compile()
    return nc


def _get_nc():
    if 'nc' not in _CACHE:
        _CACHE['nc'] = _build()
    return _CACHE['nc']


def shard_inputs(x, y, bias, wq, wk, wv, wo):
    """Build the 8 per-core input maps from full inputs."""
    scale = (H // NH) ** -0.5
    wqs = (wq * scale).astype(np.float32)
    in_maps = []
    for c in range(NCORES):
        b = c // (NCORES // B)
        g = c % (NCORES // B)
        cols = slice(g * C, (g + 1) * C)
        eb = np.exp(bias[b, 0, 0, :].astype(np.float64)).astype(np.float32)
        in_maps.append({
            'xT': np.ascontiguousarray(x[b].T),
            'yT': np.ascontiguousarray(y[b].T),
            'wq': np.ascontiguousarray(wqs[:, cols]),
            'wk': np.ascontiguousarray(wk[:, cols]),
            'wv': np.ascontiguousarray(wv[:, cols]),
            'wo': np.ascontiguousarray(wo[cols, :]),
            'ebias': np.ascontiguousarray(eb.reshape(SK, 128).T),
        })
    return in_maps


def kernel(x, y, bias, wq, wk, wv, wo, _trace=False):
    x, y, bias = np.asarray(x), np.asarray(y), np.asarray(bias)
    wq, wk, wv, wo = (np.asarray(t) for t in (wq, wk, wv, wo))
    nc = _get_nc()
    in_maps = shard_inputs(x, y, bias, wq, wk, wv, wo)
    kw = {}
    if _trace:
        kw = dict(trace=True, stitch_traces=False)
    res = bass_utils.run_bass_kernel_spmd(nc, in_maps, core_ids=list(range(NCORES)), **kw)
    full = np.zeros((B, S, H), dtype=np.float64)
    for c in range(NCORES):
        full[c // (NCORES // B)] += res.results[c]['out'].astype(np.float64)
    if _trace:
        _CACHE['last_results'] = res
    return full.astype(np.float32)


# revision 8
# speedup vs baseline: 1.2870x; 1.2870x over previous
"""Multi-head attention (B=2, S=2048, H=1024, 16 heads) on 8 TRN2 NeuronCores.

Sharding: data parallel on batch (2) x tensor parallel on heads (4 heads/core,
Megatron column-split qkv, row-split wo). Host pre-transposes x/y, pre-scales
wq by dh^-0.5, and sum-reduces the 4 partial outputs per batch element.

Per-core kernel (all matmuls in fp32r, 1 cycle/row):
  Phase A: QT/KT in [head-pair-dims(128) x S] transposed layout, V in natural
           [S x dims] layout with a fused ones column (scaled by exp(bias) so
           the additive attention bias is exact).
  Phase B: per 512-wide q-block and head pair: row-tiled (2-head packed)
           QK^T -> logitsT psum [128,1024] -> one ACT exp per pair ->
           PV matmul with fused denominator row -> gpsimd partition_broadcast
           normalize -> pair-stacked output projection.
"""
import sys
sys.path.insert(0, '/opt/trn_rl_repo')
from contextlib import ExitStack

import numpy as np

import concourse.bacc as bacc
import concourse.tile as tile
from concourse import mybir
from concourse import bass_utils

B, S, H, NH = 2, 2048, 1024, 16
DH = H // NH            # 64
NCORES = 8
HPC = NH // (NCORES // B)   # 4 heads per core
C = HPC * DH            # 256 projected cols per core
KT_H = H // 128         # 8 contraction tiles over H
SK = S // 128           # 16 s-subtiles
JBLK = 512
NJ = S // JBLK          # 4 q-blocks
F32 = mybir.dt.float32
F32R = mybir.dt.float32r
BF16 = mybir.dt.bfloat16

_CACHE = {}
_DEBUG = False


def _build():
    nc = bacc.Bacc('TRN2', debug=False, num_devices=NCORES)
    xT = nc.dram_tensor('xT', [H, S], F32R, kind='ExternalInput')
    yT = nc.dram_tensor('yT', [H, S], F32R, kind='ExternalInput')
    wq = nc.dram_tensor('wq', [H, C], F32R, kind='ExternalInput')
    wk = nc.dram_tensor('wk', [H, C], F32R, kind='ExternalInput')
    wv = nc.dram_tensor('wv', [H, C], F32R, kind='ExternalInput')
    wo = nc.dram_tensor('wo', [C, H], F32R, kind='ExternalInput')
    ebias = nc.dram_tensor('ebias', [128, SK], F32, kind='ExternalInput')
    out = nc.dram_tensor('out', [S, H], F32, kind='ExternalOutput')
    dbg = {}
    if _DEBUG:
        for nm, shp in [('d_qt', [128, S]), ('d_kt', [128, S]), ('d_v', [128, HPC * (DH + 1)]),
                        ('d_ex', [128, 2 * JBLK]), ('d_raw', [128, JBLK]), ('d_ctx', [128, JBLK])]:
            dbg[nm] = nc.dram_tensor(nm, shp, F32, kind='ExternalOutput')

    with tile.TileContext(nc) as tc, ExitStack() as ctx:
        res = ctx.enter_context(tc.tile_pool(name='res', bufs=1))
        stream = ctx.enter_context(tc.tile_pool(name='stream', bufs=3))
        expool = ctx.enter_context(tc.tile_pool(name='expool', bufs=3))
        ctxpool = ctx.enter_context(tc.tile_pool(name='ctxpool', bufs=2))
        small = ctx.enter_context(tc.tile_pool(name='small', bufs=2))
        outpool = ctx.enter_context(tc.tile_pool(name='outpool', bufs=3))

        # ---- resident weights ----
        wq_r = res.tile([128, KT_H, C], F32R, tag='wq')
        wk_r = res.tile([128, KT_H, C], F32R, tag='wk')
        wv_r = res.tile([128, KT_H, C], F32R, tag='wv')
        nc.sync.dma_start(out=wq_r, in_=wq.ap().rearrange('(t p) c -> p t c', p=128))
        nc.sync.dma_start(out=wk_r, in_=wk.ap().rearrange('(t p) c -> p t c', p=128))
        nc.sync.dma_start(out=wv_r, in_=wv.ap().rearrange('(t p) c -> p t c', p=128))
        wo_r = res.tile([128, 2, H], F32R, tag='wo')
        nc.sync.dma_start(out=wo_r, in_=wo.ap().rearrange('(t p) n -> p t n', p=128))
        eb = res.tile([128, SK], F32, tag='eb')
        nc.sync.dma_start(out=eb, in_=ebias.ap())
        ones4 = res.tile([128, HPC, 1], F32, tag='ones4')
        nc.vector.memset(ones4, 1.0)

        # ---- resident activations ----
        QT = [res.tile([128, S], BF16, tag=f'qt{p}', name=f'qt{p}') for p in range(2)]
        KTs = [res.tile([128, S], BF16, tag=f'kt{p}', name=f'kt{p}') for p in range(2)]
        # V tiles: [s-subtile 128, 4 heads x (64 v-dims + 1 e^bias col)]
        v_sb = [res.tile([128, HPC, DH + 1], BF16, tag=f'v{i}', name=f'v{i}') for i in range(SK)]

        xT_ap, yT_ap = xT.ap(), yT.ap()

        # ---- Phase A: projections ----
        actx = ExitStack()
        ps_proj = actx.enter_context(tc.tile_pool(name='ps_proj', bufs=1, space='PSUM'))
        for j4 in range(NJ):
            js = slice(j4 * JBLK, (j4 + 1) * JBLK)
            psq = [ps_proj.tile([128, JBLK], F32, tag='psq', bufs=2, name=f'psq{j4}_{i}') for i in range(2)]
            psk = [ps_proj.tile([128, JBLK], F32, tag='psk', bufs=2, name=f'psk{j4}_{i}') for i in range(2)]
            psv = [ps_proj.tile([128, C], F32, tag='psv', bufs=4, name=f'psv{j4}_{i}') for i in range(4)]
            for k in range(KT_H):
                ks = slice(k * 128, (k + 1) * 128)
                xt = stream.tile([128, JBLK], F32R, tag='xt')
                yt = stream.tile([128, JBLK], F32R, tag='yt')
                nc.sync.dma_start(out=xt, in_=xT_ap[ks, js])
                nc.sync.dma_start(out=yt, in_=yT_ap[ks, js])
                for p in range(2):
                    cs = slice(p * 128, (p + 1) * 128)
                    nc.tensor.matmul(psq[p], wq_r[:, k, cs], xt,
                                     start=(k == 0), stop=(k == KT_H - 1))
                    nc.tensor.matmul(psk[p], wk_r[:, k, cs], yt,
                                     start=(k == 0), stop=(k == KT_H - 1))
                for m in range(4):
                    nc.tensor.matmul(psv[m], yt[:, m * 128:(m + 1) * 128], wv_r[:, k, :],
                                     start=(k == 0), stop=(k == KT_H - 1))
            for p in range(2):
                nc.vector.tensor_copy(QT[p][:, js], psq[p])
                nc.vector.tensor_copy(KTs[p][:, js], psk[p])
            for m in range(4):
                sub = j4 * 4 + m
                nc.vector.tensor_scalar_mul(
                    v_sb[sub][:, :, 0:DH],
                    psv[m].rearrange('p (h c) -> p h c', h=HPC),
                    eb[:, sub:sub + 1])
                nc.vector.tensor_scalar_mul(v_sb[sub][:, :, DH:DH + 1], ones4,
                                            eb[:, sub:sub + 1])

        if _DEBUG:
            dq = outpool.tile([128, S], F32, tag='dq')
            nc.vector.tensor_copy(dq, QT[0])
            nc.sync.dma_start(out=dbg['d_qt'].ap(), in_=dq)
            dk = outpool.tile([128, S], F32, tag='dk')
            nc.vector.tensor_copy(dk, KTs[0])
            nc.sync.dma_start(out=dbg['d_kt'].ap(), in_=dk)
            dv = outpool.tile([128, HPC * (DH + 1)], F32, tag='dv')
            nc.vector.tensor_copy(dv, v_sb[0].rearrange('p h c -> p (h c)'))
            nc.sync.dma_start(out=dbg['d_v'].ap(), in_=dv)
        actx.close()

        # ---- Phase B: attention + output projection ----
        ps_qk = ctx.enter_context(tc.tile_pool(name='ps_qk', bufs=2, space='PSUM'))
        ps_pv = ctx.enter_context(tc.tile_pool(name='ps_pv', bufs=4, space='PSUM'))
        pend_out = None  # deferred out-proj emission for previous J

        def emit_out(J, ctx_tiles):
            for m in range(4):
                ms = slice(m * 128, (m + 1) * 128)
                for n in range(2):
                    ns = slice(n * JBLK, (n + 1) * JBLK)
                    pso = ps_pv.tile([128, JBLK], F32, tag='pv')
                    for p in range(2):
                        nc.tensor.matmul(pso, ctx_tiles[p][:, ms], wo_r[:, p, ns],
                                         start=(p == 0), stop=(p == 1))
                    ob = outpool.tile([128, JBLK], F32, tag='ob')
                    nc.vector.tensor_copy(ob, pso)
                    nc.sync.dma_start(out=out.ap()[J * JBLK + m * 128:
                                                   J * JBLK + (m + 1) * 128, ns],
                                      in_=ob)

        for J in range(NJ):
            js = slice(J * JBLK, (J + 1) * JBLK)
            ctx_tiles = []
            for p in range(2):
                pv0 = ps_pv.tile([128, JBLK], F32, tag='pv')
                pv1 = ps_pv.tile([128, JBLK], F32, tag='pv')
                for kk in range(SK):
                    kks = slice(kk * 128, (kk + 1) * 128)
                    psl = ps_qk.tile([128, 2 * JBLK], F32, tag='qk')
                    nc.tensor.matmul(psl[:, 0:JBLK],
                                     KTs[p][0:64, kks], QT[p][0:64, js],
                                     start=True, stop=True, tile_position=(0, 0))
                    nc.tensor.matmul(psl[:, JBLK:2 * JBLK],
                                     KTs[p][64:128, kks], QT[p][64:128, js],
                                     start=True, stop=True, tile_position=(64, 0))
                    ex = expool.tile([128, 2 * JBLK], BF16, tag='ex')
                    nc.scalar.activation(ex, psl, mybir.ActivationFunctionType.Exp)
                    if _DEBUG and J == 0 and p == 0 and kk == 0:
                        de = outpool.tile([128, 2 * JBLK], F32, tag='de')
                        nc.vector.tensor_copy(de, ex)
                        nc.sync.dma_start(out=dbg['d_ex'].ap(), in_=de)
                    for hh, pv in enumerate((pv0, pv1)):
                        hcol = 2 * p + hh
                        nc.tensor.matmul(
                            pv[0:DH + 1, :],
                            v_sb[kk][:, hcol, :],
                            ex[:, hh * JBLK:(hh + 1) * JBLK],
                            start=(kk == 0), stop=(kk == SK - 1))
                # normalize: ctxT[d, q] * (1/denom[q]) via partition broadcast
                ct = ctxpool.tile([128, JBLK], F32R, tag=f'ctx{p}')
                for hh, pv in enumerate((pv0, pv1)):
                    # single eviction frees the PSUM slot; normalize from SBUF
                    rawct = small.tile([128, JBLK], F32, tag='rawct')
                    nc.vector.tensor_copy(rawct, pv)
                    if _DEBUG and J == 0 and p == 0 and hh == 0:
                        dr = outpool.tile([128, JBLK], F32, tag='dr')
                        nc.vector.tensor_copy(dr[0:DH + 1, :], rawct[0:DH + 1, :])
                        nc.sync.dma_start(out=dbg['d_raw'].ap(), in_=dr)
                    rec = small.tile([128, JBLK], F32, tag='rec')
                    nc.vector.reciprocal_approx_fast(rec, rawct)
                    bcs = small.tile([128, JBLK], F32, tag='bcs')
                    nc.sync.dma_start(out=bcs[0:1, :], in_=rec[DH:DH + 1, :])
                    bc = small.tile([128, JBLK], F32, tag='bc')
                    nc.gpsimd.partition_broadcast(bc[0:DH, :], bcs[0:1, :])
                    if hh == 0:
                        nc.vector.tensor_mul(ct[0:DH, :], rawct[0:DH, :], bc[0:DH, :])
                    else:
                        tmp = small.tile([128, JBLK], F32R, tag='tmp')
                        nc.vector.tensor_mul(tmp[0:DH, :], rawct[0:DH, :], bc[0:DH, :])
                        nc.sync.dma_start(out=ct[DH:128, :], in_=tmp[0:DH, :])
                if _DEBUG and J == 0 and p == 0:
                    dc = outpool.tile([128, JBLK], F32, tag='dc')
                    nc.vector.tensor_copy(dc, ct)
                    nc.sync.dma_start(out=dbg['d_ctx'].ap(), in_=dc)
                ctx_tiles.append(ct)
                if p == 0 and pend_out is not None:
                    emit_out(*pend_out)
                    pend_out = None
            pend_out = (J, ctx_tiles)
        emit_out(*pend_out)

    nc.compile()
    return nc


def _get_nc():
    if 'nc' not in _CACHE:
        _CACHE['nc'] = _build()
    return _CACHE['nc']


def shard_inputs(x, y, bias, wq, wk, wv, wo):
    """Build the 8 per-core input maps from full inputs."""
    scale = (H // NH) ** -0.5
    wqs = (wq * scale).astype(np.float32)
    in_maps = []
    for c in range(NCORES):
        b = c // (NCORES // B)
        g = c % (NCORES // B)
        cols = slice(g * C, (g + 1) * C)
        eb = np.exp(bias[b, 0, 0, :].astype(np.float64)).astype(np.float32)
        in_maps.append({
            'xT': np.ascontiguousarray(x[b].T),
            'yT': np.ascontiguousarray(y[b].T),
            'wq': np.ascontiguousarray(wqs[:, cols]),
            'wk': np.ascontiguousarray(wk[:, cols]),
            'wv': np.ascontiguousarray(wv[:, cols]),
            'wo': np.ascontiguousarray(wo[cols, :]),
            'ebias': np.ascontiguousarray(eb.reshape(SK, 128).T),
        })
    return in_maps


def kernel(x, y, bias, wq, wk, wv, wo, _trace=False):
    x, y, bias = np.asarray(x), np.asarray(y), np.asarray(bias)
    wq, wk, wv, wo = (np.asarray(t) for t in (wq, wk, wv, wo))
    nc = _get_nc()
    in_maps = shard_inputs(x, y, bias, wq, wk, wv, wo)
    kw = {}
    if _trace:
        kw = dict(trace=True, stitch_traces=False)
    res = bass_utils.run_bass_kernel_spmd(nc, in_maps, core_ids=list(range(NCORES)), **kw)
    full = np.zeros((B, S, H), dtype=np.float64)
    for c in range(NCORES):
        full[c // (NCORES // B)] += res.results[c]['out'].astype(np.float64)
    if _trace:
        _CACHE['last_results'] = res
    return full.astype(np.float32)


# revision 10
# speedup vs baseline: 1.3853x; 1.0764x over previous
"""Multi-head attention (B=2, S=2048, H=1024, 16 heads) on 8 TRN2 NeuronCores.

Sharding: data parallel on batch (2) x tensor parallel on heads (4 heads/core,
Megatron column-split qkv, row-split wo). Host pre-transposes x/y, pre-scales
wq by dh^-0.5, and sum-reduces the 4 partial outputs per batch element.

Per-core kernel (all matmuls in fp32r, 1 cycle/row):
  Phase A: QT/KT in [head-pair-dims(128) x S] transposed layout, V in natural
           [S x dims] layout with a fused ones column (scaled by exp(bias) so
           the additive attention bias is exact).
  Phase B: per 512-wide q-block and head pair: row-tiled (2-head packed)
           QK^T -> logitsT psum [128,1024] -> one ACT exp per pair ->
           PV matmul with fused denominator row -> gpsimd partition_broadcast
           normalize -> pair-stacked output projection.
"""
import sys
sys.path.insert(0, '/opt/trn_rl_repo')
from contextlib import ExitStack

import numpy as np
import ml_dtypes

import concourse.bacc as bacc
import concourse.tile as tile
from concourse import mybir
from concourse import bass_utils

B, S, H, NH = 2, 2048, 1024, 16
DH = H // NH            # 64
NCORES = 8
HPC = NH // (NCORES // B)   # 4 heads per core
C = HPC * DH            # 256 projected cols per core
KT_H = H // 128         # 8 contraction tiles over H
SK = S // 128           # 16 s-subtiles
JBLK = 512
NJ = S // JBLK          # 4 q-blocks
F32 = mybir.dt.float32
F32R = mybir.dt.float32r
BF16 = mybir.dt.bfloat16

_CACHE = {}
_DEBUG = False


def _build():
    nc = bacc.Bacc('TRN2', debug=False, num_devices=NCORES)
    xT = nc.dram_tensor('xT', [H, S], BF16, kind='ExternalInput')
    yT = nc.dram_tensor('yT', [H, S], BF16, kind='ExternalInput')
    wq = nc.dram_tensor('wq', [H, C], BF16, kind='ExternalInput')
    wk = nc.dram_tensor('wk', [H, C], BF16, kind='ExternalInput')
    wv = nc.dram_tensor('wv', [H, C], BF16, kind='ExternalInput')
    wo = nc.dram_tensor('wo', [C, H], F32R, kind='ExternalInput')
    ebias = nc.dram_tensor('ebias', [128, SK], F32, kind='ExternalInput')
    out = nc.dram_tensor('out', [S, H], F32, kind='ExternalOutput')
    dbg = {}
    if _DEBUG:
        for nm, shp in [('d_qt', [128, S]), ('d_kt', [128, S]), ('d_v', [128, HPC * (DH + 1)]),
                        ('d_ex', [128, 2 * JBLK]), ('d_raw', [128, JBLK]), ('d_ctx', [128, JBLK])]:
            dbg[nm] = nc.dram_tensor(nm, shp, F32, kind='ExternalOutput')

    with tile.TileContext(nc) as tc, ExitStack() as ctx:
        res = ctx.enter_context(tc.tile_pool(name='res', bufs=1))
        expool = ctx.enter_context(tc.tile_pool(name='expool', bufs=3))
        ctxpool = ctx.enter_context(tc.tile_pool(name='ctxpool', bufs=2))
        small = ctx.enter_context(tc.tile_pool(name='small', bufs=2))
        outpool = ctx.enter_context(tc.tile_pool(name='outpool', bufs=3))

        # ---- resident weights ----
        wq_r = res.tile([128, KT_H, C], BF16, tag='wq')
        wk_r = res.tile([128, KT_H, C], BF16, tag='wk')
        wv_r = res.tile([128, KT_H, C], BF16, tag='wv')
        nc.sync.dma_start(out=wq_r, in_=wq.ap().rearrange('(t p) c -> p t c', p=128))
        nc.sync.dma_start(out=wk_r, in_=wk.ap().rearrange('(t p) c -> p t c', p=128))
        nc.sync.dma_start(out=wv_r, in_=wv.ap().rearrange('(t p) c -> p t c', p=128))
        wo_r = res.tile([128, 2, H], F32R, tag='wo')
        nc.sync.dma_start(out=wo_r, in_=wo.ap().rearrange('(t p) n -> p t n', p=128))
        eb = res.tile([128, SK], F32, tag='eb')
        nc.sync.dma_start(out=eb, in_=ebias.ap())
        ones4 = res.tile([128, HPC, 1], F32, tag='ones4')
        nc.vector.memset(ones4, 1.0)

        # ---- resident activations ----
        QT = [res.tile([128, S], BF16, tag=f'qt{p}', name=f'qt{p}') for p in range(2)]
        KTs = [res.tile([128, S], BF16, tag=f'kt{p}', name=f'kt{p}') for p in range(2)]
        # V tiles: [s-subtile 128, 4 heads x (64 v-dims + 1 e^bias col)]
        v_sb = [res.tile([128, HPC, DH + 1], BF16, tag=f'v{i}', name=f'v{i}') for i in range(SK)]

        xT_ap, yT_ap = xT.ap(), yT.ap()
        xts = [res.tile([128, S], BF16, tag=f'xts{k}', name=f'xts{k}') for k in range(KT_H)]
        yts = [res.tile([128, S], BF16, tag=f'yts{k}', name=f'yts{k}') for k in range(KT_H)]
        for k in range(KT_H):
            ks = slice(k * 128, (k + 1) * 128)
            nc.sync.dma_start(out=xts[k], in_=xT_ap[ks, :])
            nc.sync.dma_start(out=yts[k], in_=yT_ap[ks, :])

        # ---- Phase A: projections ----
        actx = ExitStack()
        ps_proj = actx.enter_context(tc.tile_pool(name='ps_proj', bufs=1, space='PSUM'))
        for j4 in range(NJ):
            js = slice(j4 * JBLK, (j4 + 1) * JBLK)
            psq = [ps_proj.tile([128, JBLK], F32, tag='psq', bufs=2, name=f'psq{j4}_{i}') for i in range(2)]
            psk = [ps_proj.tile([128, JBLK], F32, tag='psk', bufs=2, name=f'psk{j4}_{i}') for i in range(2)]
            psv = [ps_proj.tile([128, C], F32, tag='psv', bufs=4, name=f'psv{j4}_{i}') for i in range(4)]
            for k in range(KT_H):
                xt = xts[k][:, js]
                yt = yts[k][:, js]
                for p in range(2):
                    cs = slice(p * 128, (p + 1) * 128)
                    nc.tensor.matmul(psq[p], wq_r[:, k, cs], xt,
                                     start=(k == 0), stop=(k == KT_H - 1))
                    nc.tensor.matmul(psk[p], wk_r[:, k, cs], yt,
                                     start=(k == 0), stop=(k == KT_H - 1))
                for m in range(4):
                    nc.tensor.matmul(psv[m], yt[:, m * 128:(m + 1) * 128], wv_r[:, k, :],
                                     start=(k == 0), stop=(k == KT_H - 1))
            for p in range(2):
                nc.vector.tensor_copy(QT[p][:, js], psq[p])
                nc.vector.tensor_copy(KTs[p][:, js], psk[p])
            for m in range(4):
                sub = j4 * 4 + m
                nc.vector.tensor_scalar_mul(
                    v_sb[sub][:, :, 0:DH],
                    psv[m].rearrange('p (h c) -> p h c', h=HPC),
                    eb[:, sub:sub + 1])
                nc.vector.tensor_scalar_mul(v_sb[sub][:, :, DH:DH + 1], ones4,
                                            eb[:, sub:sub + 1])

        if _DEBUG:
            dq = outpool.tile([128, S], F32, tag='dq')
            nc.vector.tensor_copy(dq, QT[0])
            nc.sync.dma_start(out=dbg['d_qt'].ap(), in_=dq)
            dk = outpool.tile([128, S], F32, tag='dk')
            nc.vector.tensor_copy(dk, KTs[0])
            nc.sync.dma_start(out=dbg['d_kt'].ap(), in_=dk)
            dv = outpool.tile([128, HPC * (DH + 1)], F32, tag='dv')
            nc.vector.tensor_copy(dv, v_sb[0].rearrange('p h c -> p (h c)'))
            nc.sync.dma_start(out=dbg['d_v'].ap(), in_=dv)
        actx.close()

        # ---- Phase B: attention + output projection ----
        ps_qk = ctx.enter_context(tc.tile_pool(name='ps_qk', bufs=2, space='PSUM'))
        ps_pv = ctx.enter_context(tc.tile_pool(name='ps_pv', bufs=4, space='PSUM'))
        pend_out = None  # deferred out-proj emission for previous J

        def emit_out(J, ctx_tiles):
            for m in range(4):
                ms = slice(m * 128, (m + 1) * 128)
                for n in range(2):
                    ns = slice(n * JBLK, (n + 1) * JBLK)
                    pso = ps_pv.tile([128, JBLK], F32, tag='pv')
                    for p in range(2):
                        nc.tensor.matmul(pso, ctx_tiles[p][:, ms], wo_r[:, p, ns],
                                         start=(p == 0), stop=(p == 1))
                    ob = outpool.tile([128, JBLK], F32, tag='ob')
                    nc.vector.tensor_copy(ob, pso)
                    nc.sync.dma_start(out=out.ap()[J * JBLK + m * 128:
                                                   J * JBLK + (m + 1) * 128, ns],
                                      in_=ob)

        for J in range(NJ):
            js = slice(J * JBLK, (J + 1) * JBLK)
            ctx_tiles = []
            for p in range(2):
                pv0 = ps_pv.tile([128, JBLK], F32, tag='pv')
                pv1 = ps_pv.tile([128, JBLK], F32, tag='pv')
                for kk in range(SK):
                    kks = slice(kk * 128, (kk + 1) * 128)
                    psl = ps_qk.tile([128, 2 * JBLK], F32, tag='qk')
                    nc.tensor.matmul(psl[:, 0:JBLK],
                                     KTs[p][0:64, kks], QT[p][0:64, js],
                                     start=True, stop=True, tile_position=(0, 0))
                    nc.tensor.matmul(psl[:, JBLK:2 * JBLK],
                                     KTs[p][64:128, kks], QT[p][64:128, js],
                                     start=True, stop=True, tile_position=(64, 0))
                    ex = expool.tile([128, 2 * JBLK], BF16, tag='ex')
                    nc.scalar.activation(ex, psl, mybir.ActivationFunctionType.Exp)
                    if _DEBUG and J == 0 and p == 0 and kk == 0:
                        de = outpool.tile([128, 2 * JBLK], F32, tag='de')
                        nc.vector.tensor_copy(de, ex)
                        nc.sync.dma_start(out=dbg['d_ex'].ap(), in_=de)
                    for hh, pv in enumerate((pv0, pv1)):
                        hcol = 2 * p + hh
                        nc.tensor.matmul(
                            pv[0:DH + 1, :],
                            v_sb[kk][:, hcol, :],
                            ex[:, hh * JBLK:(hh + 1) * JBLK],
                            start=(kk == 0), stop=(kk == SK - 1))
                # normalize: ctxT[d, q] * (1/denom[q]) via partition broadcast
                ct = ctxpool.tile([128, JBLK], F32R, tag=f'ctx{p}')
                for hh, pv in enumerate((pv0, pv1)):
                    # single eviction frees the PSUM slot; normalize from SBUF
                    rawct = small.tile([128, JBLK], F32, tag='rawct')
                    nc.vector.tensor_copy(rawct[0:DH + 1, :], pv[0:DH + 1, :])
                    if _DEBUG and J == 0 and p == 0 and hh == 0:
                        dr = outpool.tile([128, JBLK], F32, tag='dr')
                        nc.vector.tensor_copy(dr[0:DH + 1, :], rawct[0:DH + 1, :])
                        nc.sync.dma_start(out=dbg['d_raw'].ap(), in_=dr)
                    rec = small.tile([128, JBLK], F32, tag='rec')
                    nc.vector.reciprocal_approx_fast(rec[0:DH + 1, :], rawct[0:DH + 1, :])
                    bcs = small.tile([128, JBLK], F32, tag='bcs')
                    nc.sync.dma_start(out=bcs[0:1, :], in_=rec[DH:DH + 1, :])
                    bc = small.tile([128, JBLK], F32, tag='bc')
                    nc.gpsimd.partition_broadcast(bc[0:DH, :], bcs[0:1, :])
                    if hh == 0:
                        nc.vector.tensor_mul(ct[0:DH, :], rawct[0:DH, :], bc[0:DH, :])
                    else:
                        tmp = small.tile([128, JBLK], F32R, tag='tmp')
                        nc.vector.tensor_mul(tmp[0:DH, :], rawct[0:DH, :], bc[0:DH, :])
                        nc.sync.dma_start(out=ct[DH:128, :], in_=tmp[0:DH, :])
                if _DEBUG and J == 0 and p == 0:
                    dc = outpool.tile([128, JBLK], F32, tag='dc')
                    nc.vector.tensor_copy(dc, ct)
                    nc.sync.dma_start(out=dbg['d_ctx'].ap(), in_=dc)
                ctx_tiles.append(ct)
                if p == 0 and pend_out is not None:
                    emit_out(*pend_out)
                    pend_out = None
            pend_out = (J, ctx_tiles)
        emit_out(*pend_out)

    nc.compile()
    return nc


def _get_nc():
    if 'nc' not in _CACHE:
        _CACHE['nc'] = _build()
    return _CACHE['nc']


def shard_inputs(x, y, bias, wq, wk, wv, wo):
    """Build the 8 per-core input maps from full inputs."""
    scale = (H // NH) ** -0.5
    wqs = (wq * scale).astype(np.float32)
    in_maps = []
    for c in range(NCORES):
        b = c // (NCORES // B)
        g = c % (NCORES // B)
        cols = slice(g * C, (g + 1) * C)
        eb = np.exp(bias[b, 0, 0, :].astype(np.float64)).astype(np.float32)
        bf = ml_dtypes.bfloat16
        in_maps.append({
            'xT': np.ascontiguousarray(x[b].T.astype(bf)),
            'yT': np.ascontiguousarray(y[b].T.astype(bf)),
            'wq': np.ascontiguousarray(wqs[:, cols].astype(bf)),
            'wk': np.ascontiguousarray(wk[:, cols].astype(bf)),
            'wv': np.ascontiguousarray(wv[:, cols].astype(bf)),
            'wo': np.ascontiguousarray(wo[cols, :]),
            'ebias': np.ascontiguousarray(eb.reshape(SK, 128).T),
        })
    return in_maps


def kernel(x, y, bias, wq, wk, wv, wo, _trace=False):
    x, y, bias = np.asarray(x), np.asarray(y), np.asarray(bias)
    wq, wk, wv, wo = (np.asarray(t) for t in (wq, wk, wv, wo))
    nc = _get_nc()
    in_maps = shard_inputs(x, y, bias, wq, wk, wv, wo)
    kw = {}
    if _trace:
        kw = dict(trace=True, stitch_traces=False)
    res = bass_utils.run_bass_kernel_spmd(nc, in_maps, core_ids=list(range(NCORES)), **kw)
    full = np.zeros((B, S, H), dtype=np.float64)
    for c in range(NCORES):
        full[c // (NCORES // B)] += res.results[c]['out'].astype(np.float64)
    if _trace:
        _CACHE['last_results'] = res
    return full.astype(np.float32)


# revision 11
# speedup vs baseline: 1.4180x; 1.0236x over previous
"""Multi-head attention (B=2, S=2048, H=1024, 16 heads) on 8 TRN2 NeuronCores.

Sharding: data parallel on batch (2) x tensor parallel on heads (4 heads/core,
Megatron column-split qkv, row-split wo). Host pre-transposes x/y, pre-scales
wq by dh^-0.5, and sum-reduces the 4 partial outputs per batch element.

Per-core kernel (all matmuls in fp32r, 1 cycle/row):
  Phase A: QT/KT in [head-pair-dims(128) x S] transposed layout, V in natural
           [S x dims] layout with a fused ones column (scaled by exp(bias) so
           the additive attention bias is exact).
  Phase B: per 512-wide q-block and head pair: row-tiled (2-head packed)
           QK^T -> logitsT psum [128,1024] -> one ACT exp per pair ->
           PV matmul with fused denominator row -> gpsimd partition_broadcast
           normalize -> pair-stacked output projection.
"""
import sys
sys.path.insert(0, '/opt/trn_rl_repo')
from contextlib import ExitStack

import numpy as np
import ml_dtypes

import concourse.bacc as bacc
import concourse.tile as tile
from concourse import mybir
from concourse import bass_utils

B, S, H, NH = 2, 2048, 1024, 16
DH = H // NH            # 64
NCORES = 8
HPC = NH // (NCORES // B)   # 4 heads per core
C = HPC * DH            # 256 projected cols per core
KT_H = H // 128         # 8 contraction tiles over H
SK = S // 128           # 16 s-subtiles
JBLK = 512
NJ = S // JBLK          # 4 q-blocks
F32 = mybir.dt.float32
F32R = mybir.dt.float32r
BF16 = mybir.dt.bfloat16

_CACHE = {}
_DEBUG = False


def _build():
    nc = bacc.Bacc('TRN2', debug=False, num_devices=NCORES)
    xT = nc.dram_tensor('xT', [H, S], BF16, kind='ExternalInput')
    yT = nc.dram_tensor('yT', [H, S], BF16, kind='ExternalInput')
    wq = nc.dram_tensor('wq', [H, C], BF16, kind='ExternalInput')
    wk = nc.dram_tensor('wk', [H, C], BF16, kind='ExternalInput')
    wv = nc.dram_tensor('wv', [H, C], BF16, kind='ExternalInput')
    wo = nc.dram_tensor('wo', [C, H], BF16, kind='ExternalInput')
    ebias = nc.dram_tensor('ebias', [128, SK], F32, kind='ExternalInput')
    out = nc.dram_tensor('out', [S, H], F32, kind='ExternalOutput')
    dbg = {}
    if _DEBUG:
        for nm, shp in [('d_qt', [128, S]), ('d_kt', [128, S]), ('d_v', [128, HPC * (DH + 1)]),
                        ('d_ex', [128, 2 * JBLK]), ('d_raw', [128, JBLK]), ('d_ctx', [128, JBLK])]:
            dbg[nm] = nc.dram_tensor(nm, shp, F32, kind='ExternalOutput')

    with tile.TileContext(nc) as tc, ExitStack() as ctx:
        res = ctx.enter_context(tc.tile_pool(name='res', bufs=1))
        expool = ctx.enter_context(tc.tile_pool(name='expool', bufs=3))
        ctxpool = ctx.enter_context(tc.tile_pool(name='ctxpool', bufs=2))
        small = ctx.enter_context(tc.tile_pool(name='small', bufs=2))
        outpool = ctx.enter_context(tc.tile_pool(name='outpool', bufs=3))

        # ---- resident weights ----
        wq_r = res.tile([128, KT_H, C], BF16, tag='wq')
        wk_r = res.tile([128, KT_H, C], BF16, tag='wk')
        wv_r = res.tile([128, KT_H, C], BF16, tag='wv')
        nc.sync.dma_start(out=wq_r, in_=wq.ap().rearrange('(t p) c -> p t c', p=128))
        nc.sync.dma_start(out=wk_r, in_=wk.ap().rearrange('(t p) c -> p t c', p=128))
        nc.sync.dma_start(out=wv_r, in_=wv.ap().rearrange('(t p) c -> p t c', p=128))

        # ---- resident activations ----
        QT = [res.tile([128, S], BF16, tag=f'qt{p}', name=f'qt{p}') for p in range(2)]
        KTs = [res.tile([128, S], BF16, tag=f'kt{p}', name=f'kt{p}') for p in range(2)]
        # V tiles: [s-subtile 128, 4 heads x (64 v-dims + 1 e^bias col)]
        v_sb = [res.tile([128, HPC, DH + 1], BF16, tag=f'v{i}', name=f'v{i}') for i in range(SK)]

        xT_ap, yT_ap = xT.ap(), yT.ap()
        xts = [res.tile([128, S], BF16, tag=f'xts{k}', name=f'xts{k}') for k in range(KT_H)]
        yts = [res.tile([128, S], BF16, tag=f'yts{k}', name=f'yts{k}') for k in range(KT_H)]
        for k in range(KT_H):
            ks = slice(k * 128, (k + 1) * 128)
            nc.sync.dma_start(out=yts[k], in_=yT_ap[ks, :])
            nc.sync.dma_start(out=xts[k], in_=xT_ap[ks, :])
        wo_r = res.tile([128, 2, H], BF16, tag='wo')
        nc.sync.dma_start(out=wo_r, in_=wo.ap().rearrange('(t p) n -> p t n', p=128))
        eb = res.tile([128, SK], F32, tag='eb')
        nc.sync.dma_start(out=eb, in_=ebias.ap())
        ones4 = res.tile([128, HPC, 1], F32, tag='ones4')
        nc.vector.memset(ones4, 1.0)

        # ---- unified PSUM pools (shared across phases) ----
        ps_qk = ctx.enter_context(tc.tile_pool(name='ps_qk', bufs=2, space='PSUM'))
        ps_pv = ctx.enter_context(tc.tile_pool(name='ps_pv', bufs=4, space='PSUM'))

        # ---- Phase A: projections ----
        for j4 in range(NJ):
            js = slice(j4 * JBLK, (j4 + 1) * JBLK)
            psqk = [ps_qk.tile([128, 2 * JBLK], F32, tag='qk', name=f'psqk{j4}_{i}')
                    for i in range(2)]
            psv = [ps_pv.tile([128, JBLK], F32, tag='pv', name=f'psv{j4}_{i}')
                   for i in range(4)]
            for k in range(KT_H):
                xt = xts[k][:, js]
                yt = yts[k][:, js]
                for p in range(2):
                    cs = slice(p * 128, (p + 1) * 128)
                    nc.tensor.matmul(psqk[p][:, 0:JBLK], wq_r[:, k, cs], xt,
                                     start=(k == 0), stop=(k == KT_H - 1))
                    nc.tensor.matmul(psqk[p][:, JBLK:2 * JBLK], wk_r[:, k, cs], yt,
                                     start=(k == 0), stop=(k == KT_H - 1))
                for m in range(4):
                    nc.tensor.matmul(psv[m][:, 0:C], yt[:, m * 128:(m + 1) * 128],
                                     wv_r[:, k, :],
                                     start=(k == 0), stop=(k == KT_H - 1))
            for p in range(2):
                nc.vector.tensor_copy(QT[p][:, js], psqk[p][:, 0:JBLK])
                nc.vector.tensor_copy(KTs[p][:, js], psqk[p][:, JBLK:2 * JBLK])
            for m in range(4):
                sub = j4 * 4 + m
                nc.vector.tensor_scalar_mul(
                    v_sb[sub][:, :, 0:DH],
                    psv[m][:, 0:C].rearrange('p (h c) -> p h c', h=HPC),
                    eb[:, sub:sub + 1])
                nc.vector.tensor_scalar_mul(v_sb[sub][:, :, DH:DH + 1], ones4,
                                            eb[:, sub:sub + 1])

        if _DEBUG:
            dq = outpool.tile([128, S], F32, tag='dq')
            nc.vector.tensor_copy(dq, QT[0])
            nc.sync.dma_start(out=dbg['d_qt'].ap(), in_=dq)
            dk = outpool.tile([128, S], F32, tag='dk')
            nc.vector.tensor_copy(dk, KTs[0])
            nc.sync.dma_start(out=dbg['d_kt'].ap(), in_=dk)
            dv = outpool.tile([128, HPC * (DH + 1)], F32, tag='dv')
            nc.vector.tensor_copy(dv, v_sb[0].rearrange('p h c -> p (h c)'))
            nc.sync.dma_start(out=dbg['d_v'].ap(), in_=dv)
        # ---- Phase B: attention + output projection ----
        pend_out = None  # deferred out-proj emission for previous J

        def emit_out(J, ctx_tiles):
            for m in range(4):
                ms = slice(m * 128, (m + 1) * 128)
                for n in range(2):
                    ns = slice(n * JBLK, (n + 1) * JBLK)
                    pso = ps_pv.tile([128, JBLK], F32, tag='pv')
                    for p in range(2):
                        nc.tensor.matmul(pso, ctx_tiles[p][:, ms], wo_r[:, p, ns],
                                         start=(p == 0), stop=(p == 1))
                    ob = outpool.tile([128, JBLK], F32, tag='ob')
                    nc.vector.tensor_copy(ob, pso)
                    nc.sync.dma_start(out=out.ap()[J * JBLK + m * 128:
                                                   J * JBLK + (m + 1) * 128, ns],
                                      in_=ob)

        for J in range(NJ):
            js = slice(J * JBLK, (J + 1) * JBLK)
            ctx_tiles = []
            for p in range(2):
                pv0 = ps_pv.tile([128, JBLK], F32, tag='pv')
                pv1 = ps_pv.tile([128, JBLK], F32, tag='pv')
                for kk in range(SK):
                    kks = slice(kk * 128, (kk + 1) * 128)
                    psl = ps_qk.tile([128, 2 * JBLK], F32, tag='qk')
                    nc.tensor.matmul(psl[:, 0:JBLK],
                                     KTs[p][0:64, kks], QT[p][0:64, js],
                                     start=True, stop=True, tile_position=(0, 0))
                    nc.tensor.matmul(psl[:, JBLK:2 * JBLK],
                                     KTs[p][64:128, kks], QT[p][64:128, js],
                                     start=True, stop=True, tile_position=(64, 0))
                    ex = expool.tile([128, 2 * JBLK], BF16, tag='ex')
                    nc.scalar.activation(ex, psl, mybir.ActivationFunctionType.Exp)
                    if _DEBUG and J == 0 and p == 0 and kk == 0:
                        de = outpool.tile([128, 2 * JBLK], F32, tag='de')
                        nc.vector.tensor_copy(de, ex)
                        nc.sync.dma_start(out=dbg['d_ex'].ap(), in_=de)
                    for hh, pv in enumerate((pv0, pv1)):
                        hcol = 2 * p + hh
                        nc.tensor.matmul(
                            pv[0:DH + 1, :],
                            v_sb[kk][:, hcol, :],
                            ex[:, hh * JBLK:(hh + 1) * JBLK],
                            start=(kk == 0), stop=(kk == SK - 1))
                # normalize: ctxT[d, q] * (1/denom[q]) via partition broadcast
                ct = ctxpool.tile([128, JBLK], BF16, tag=f'ctx{p}')
                for hh, pv in enumerate((pv0, pv1)):
                    # single eviction frees the PSUM slot; normalize from SBUF
                    rawct = small.tile([128, JBLK], F32, tag='rawct')
                    nc.vector.tensor_copy(rawct[0:DH + 1, :], pv[0:DH + 1, :])
                    if _DEBUG and J == 0 and p == 0 and hh == 0:
                        dr = outpool.tile([128, JBLK], F32, tag='dr')
                        nc.vector.tensor_copy(dr[0:DH + 1, :], rawct[0:DH + 1, :])
                        nc.sync.dma_start(out=dbg['d_raw'].ap(), in_=dr)
                    rec = small.tile([128, JBLK], F32, tag='rec')
                    nc.vector.reciprocal_approx_fast(rec[0:DH + 1, :], rawct[0:DH + 1, :])
                    bcs = small.tile([128, JBLK], F32, tag='bcs')
                    nc.sync.dma_start(out=bcs[0:1, :], in_=rec[DH:DH + 1, :])
                    bc = small.tile([128, JBLK], F32, tag='bc')
                    nc.gpsimd.partition_broadcast(bc[0:DH, :], bcs[0:1, :])
                    if hh == 0:
                        nc.vector.tensor_mul(ct[0:DH, :], rawct[0:DH, :], bc[0:DH, :])
                    else:
                        tmp = small.tile([128, JBLK], BF16, tag='tmp')
                        nc.vector.tensor_mul(tmp[0:DH, :], rawct[0:DH, :], bc[0:DH, :])
                        nc.sync.dma_start(out=ct[DH:128, :], in_=tmp[0:DH, :])
                if _DEBUG and J == 0 and p == 0:
                    dc = outpool.tile([128, JBLK], F32, tag='dc')
                    nc.vector.tensor_copy(dc, ct)
                    nc.sync.dma_start(out=dbg['d_ctx'].ap(), in_=dc)
                ctx_tiles.append(ct)
                if p == 0 and pend_out is not None:
                    emit_out(*pend_out)
                    pend_out = None
            pend_out = (J, ctx_tiles)
        emit_out(*pend_out)

    nc.compile()
    return nc


def _get_nc():
    if 'nc' not in _CACHE:
        _CACHE['nc'] = _build()
    return _CACHE['nc']


def shard_inputs(x, y, bias, wq, wk, wv, wo):
    """Build the 8 per-core input maps from full inputs."""
    scale = (H // NH) ** -0.5
    wqs = (wq * scale).astype(np.float32)
    in_maps = []
    for c in range(NCORES):
        b = c // (NCORES // B)
        g = c % (NCORES // B)
        cols = slice(g * C, (g + 1) * C)
        eb = np.exp(bias[b, 0, 0, :].astype(np.float64)).astype(np.float32)
        bf = ml_dtypes.bfloat16
        in_maps.append({
            'xT': np.ascontiguousarray(x[b].T.astype(bf)),
            'yT': np.ascontiguousarray(y[b].T.astype(bf)),
            'wq': np.ascontiguousarray(wqs[:, cols].astype(bf)),
            'wk': np.ascontiguousarray(wk[:, cols].astype(bf)),
            'wv': np.ascontiguousarray(wv[:, cols].astype(bf)),
            'wo': np.ascontiguousarray(wo[cols, :].astype(bf)),
            'ebias': np.ascontiguousarray(eb.reshape(SK, 128).T),
        })
    return in_maps


def kernel(x, y, bias, wq, wk, wv, wo, _trace=False):
    x, y, bias = np.asarray(x), np.asarray(y), np.asarray(bias)
    wq, wk, wv, wo = (np.asarray(t) for t in (wq, wk, wv, wo))
    nc = _get_nc()
    in_maps = shard_inputs(x, y, bias, wq, wk, wv, wo)
    kw = {}
    if _trace:
        kw = dict(trace=True, stitch_traces=False)
    res = bass_utils.run_bass_kernel_spmd(nc, in_maps, core_ids=list(range(NCORES)), **kw)
    full = np.zeros((B, S, H), dtype=np.float64)
    for c in range(NCORES):
        full[c // (NCORES // B)] += res.results[c]['out'].astype(np.float64)
    if _trace:
        _CACHE['last_results'] = res
    return full.astype(np.float32)


# revision 12
# speedup vs baseline: 1.4671x; 1.0346x over previous
"""Multi-head attention (B=2, S=2048, H=1024, 16 heads) on 8 TRN2 NeuronCores.

Sharding: data parallel on batch (2) x tensor parallel on heads (4 heads/core,
Megatron column-split qkv, row-split wo). Host pre-transposes x/y, pre-scales
wq by dh^-0.5, and sum-reduces the 4 partial outputs per batch element.

Per-core kernel (all matmuls in fp32r, 1 cycle/row):
  Phase A: QT/KT in [head-pair-dims(128) x S] transposed layout, V in natural
           [S x dims] layout with a fused ones column (scaled by exp(bias) so
           the additive attention bias is exact).
  Phase B: per 512-wide q-block and head pair: row-tiled (2-head packed)
           QK^T -> logitsT psum [128,1024] -> one ACT exp per pair ->
           PV matmul with fused denominator row -> gpsimd partition_broadcast
           normalize -> pair-stacked output projection.
"""
import sys
sys.path.insert(0, '/opt/trn_rl_repo')
from contextlib import ExitStack

import numpy as np
import ml_dtypes

import concourse.bacc as bacc
import concourse.tile as tile
from concourse import mybir
from concourse import bass_utils

B, S, H, NH = 2, 2048, 1024, 16
DH = H // NH            # 64
NCORES = 8
HPC = NH // (NCORES // B)   # 4 heads per core
C = HPC * DH            # 256 projected cols per core
KT_H = H // 128         # 8 contraction tiles over H
SK = S // 128           # 16 s-subtiles
JBLK = 512
NJ = S // JBLK          # 4 q-blocks
F32 = mybir.dt.float32
F32R = mybir.dt.float32r
BF16 = mybir.dt.bfloat16

_CACHE = {}
_DEBUG = False


def _build():
    nc = bacc.Bacc('TRN2', debug=False, num_devices=NCORES)
    xT = nc.dram_tensor('xT', [H, S], BF16, kind='ExternalInput')
    yT = nc.dram_tensor('yT', [H, S], BF16, kind='ExternalInput')
    wq = nc.dram_tensor('wq', [H, C], BF16, kind='ExternalInput')
    wk = nc.dram_tensor('wk', [H, C], BF16, kind='ExternalInput')
    wv = nc.dram_tensor('wv', [H, C], BF16, kind='ExternalInput')
    wo = nc.dram_tensor('wo', [C, H], BF16, kind='ExternalInput')
    ebias = nc.dram_tensor('ebias', [128, SK], F32, kind='ExternalInput')
    out = nc.dram_tensor('out', [S, H], F32, kind='ExternalOutput')
    dbg = {}
    if _DEBUG:
        for nm, shp in [('d_qt', [128, S]), ('d_kt', [128, S]), ('d_v', [128, HPC * (DH + 1)]),
                        ('d_ex', [128, 2 * JBLK]), ('d_raw', [128, JBLK]), ('d_ctx', [128, JBLK])]:
            dbg[nm] = nc.dram_tensor(nm, shp, F32, kind='ExternalOutput')

    with tile.TileContext(nc) as tc, ExitStack() as ctx:
        res = ctx.enter_context(tc.tile_pool(name='res', bufs=1))
        expool = ctx.enter_context(tc.tile_pool(name='expool', bufs=3))
        ctxpool = ctx.enter_context(tc.tile_pool(name='ctxpool', bufs=2))
        small = ctx.enter_context(tc.tile_pool(name='small', bufs=2))
        outpool = ctx.enter_context(tc.tile_pool(name='outpool', bufs=3))

        # ---- resident weights ----
        wq_r = res.tile([128, KT_H, C], BF16, tag='wq')
        wk_r = res.tile([128, KT_H, C], BF16, tag='wk')
        wv_r = res.tile([128, KT_H, C], BF16, tag='wv')
        nc.sync.dma_start(out=wq_r, in_=wq.ap().rearrange('(t p) c -> p t c', p=128))
        nc.sync.dma_start(out=wk_r, in_=wk.ap().rearrange('(t p) c -> p t c', p=128))
        nc.sync.dma_start(out=wv_r, in_=wv.ap().rearrange('(t p) c -> p t c', p=128))

        # ---- resident activations ----
        QT = [res.tile([128, S], BF16, tag=f'qt{p}', name=f'qt{p}') for p in range(2)]
        KTs = [res.tile([128, S], BF16, tag=f'kt{p}', name=f'kt{p}') for p in range(2)]
        # V tiles: [s-subtile 128, 4 heads x (64 v-dims + 1 e^bias col)]
        v_sb = [res.tile([128, HPC, DH + 1], BF16, tag=f'v{i}', name=f'v{i}') for i in range(SK)]

        xT_ap, yT_ap = xT.ap(), yT.ap()
        HB = S // 2
        xts = [[res.tile([128, HB], BF16, tag=f'xts{k}_{j}', name=f'xts{k}_{j}')
                for j in range(2)] for k in range(KT_H)]
        yts = [[res.tile([128, HB], BF16, tag=f'yts{k}_{j}', name=f'yts{k}_{j}')
                for j in range(2)] for k in range(KT_H)]
        for j in range(2):
            hs = slice(j * HB, (j + 1) * HB)
            for k in range(KT_H):
                ks = slice(k * 128, (k + 1) * 128)
                nc.scalar.dma_start(out=yts[k][j], in_=yT_ap[ks, hs])
                nc.scalar.dma_start(out=xts[k][j], in_=xT_ap[ks, hs])
        wo_r = res.tile([128, 2, H], BF16, tag='wo')
        nc.sync.dma_start(out=wo_r, in_=wo.ap().rearrange('(t p) n -> p t n', p=128))
        eb = res.tile([128, SK], F32, tag='eb')
        nc.sync.dma_start(out=eb, in_=ebias.ap())
        ones4 = res.tile([128, HPC, 1], F32, tag='ones4')
        nc.vector.memset(ones4, 1.0)

        # ---- unified PSUM pools (shared across phases) ----
        ps_qk = ctx.enter_context(tc.tile_pool(name='ps_qk', bufs=2, space='PSUM'))
        ps_pv = ctx.enter_context(tc.tile_pool(name='ps_pv', bufs=4, space='PSUM'))

        # ---- Phase A: projections ----
        for j4 in range(NJ):
            js = slice(j4 * JBLK, (j4 + 1) * JBLK)
            psqk = [ps_qk.tile([128, 2 * JBLK], F32, tag='qk', name=f'psqk{j4}_{i}')
                    for i in range(2)]
            psv = [ps_pv.tile([128, JBLK], F32, tag='pv', name=f'psv{j4}_{i}')
                   for i in range(4)]
            for k in range(KT_H):
                hj = slice((j4 % 2) * JBLK, (j4 % 2 + 1) * JBLK)
                xt = xts[k][j4 // 2][:, hj]
                yt = yts[k][j4 // 2][:, hj]
                for p in range(2):
                    cs = slice(p * 128, (p + 1) * 128)
                    nc.tensor.matmul(psqk[p][:, 0:JBLK], wq_r[:, k, cs], xt,
                                     start=(k == 0), stop=(k == KT_H - 1))
                    nc.tensor.matmul(psqk[p][:, JBLK:2 * JBLK], wk_r[:, k, cs], yt,
                                     start=(k == 0), stop=(k == KT_H - 1))
                for m in range(4):
                    nc.tensor.matmul(psv[m][:, 0:C], yt[:, m * 128:(m + 1) * 128],
                                     wv_r[:, k, :],
                                     start=(k == 0), stop=(k == KT_H - 1))
            for p in range(2):
                nc.vector.tensor_copy(QT[p][:, js], psqk[p][:, 0:JBLK])
                nc.vector.tensor_copy(KTs[p][:, js], psqk[p][:, JBLK:2 * JBLK])
            for m in range(4):
                sub = j4 * 4 + m
                nc.vector.tensor_scalar_mul(
                    v_sb[sub][:, :, 0:DH],
                    psv[m][:, 0:C].rearrange('p (h c) -> p h c', h=HPC),
                    eb[:, sub:sub + 1])
                nc.vector.tensor_scalar_mul(v_sb[sub][:, :, DH:DH + 1], ones4,
                                            eb[:, sub:sub + 1])

        if _DEBUG:
            dq = outpool.tile([128, S], F32, tag='dq')
            nc.vector.tensor_copy(dq, QT[0])
            nc.sync.dma_start(out=dbg['d_qt'].ap(), in_=dq)
            dk = outpool.tile([128, S], F32, tag='dk')
            nc.vector.tensor_copy(dk, KTs[0])
            nc.sync.dma_start(out=dbg['d_kt'].ap(), in_=dk)
            dv = outpool.tile([128, HPC * (DH + 1)], F32, tag='dv')
            nc.vector.tensor_copy(dv, v_sb[0].rearrange('p h c -> p (h c)'))
            nc.sync.dma_start(out=dbg['d_v'].ap(), in_=dv)
        # ---- Phase B: attention + output projection ----
        pend_out = None  # deferred out-proj emission for previous J

        def emit_out(J, ctx_tiles):
            for m in range(4):
                ms = slice(m * 128, (m + 1) * 128)
                for n in range(2):
                    ns = slice(n * JBLK, (n + 1) * JBLK)
                    pso = ps_pv.tile([128, JBLK], F32, tag='pv')
                    for p in range(2):
                        nc.tensor.matmul(pso, ctx_tiles[p][:, ms], wo_r[:, p, ns],
                                         start=(p == 0), stop=(p == 1))
                    ob = outpool.tile([128, JBLK], F32, tag='ob')
                    nc.vector.tensor_copy(ob, pso)
                    nc.sync.dma_start(out=out.ap()[J * JBLK + m * 128:
                                                   J * JBLK + (m + 1) * 128, ns],
                                      in_=ob)

        for J in range(NJ):
            js = slice(J * JBLK, (J + 1) * JBLK)
            ctx_tiles = []
            for p in range(2):
                pv0 = ps_pv.tile([128, JBLK], F32, tag='pv')
                pv1 = ps_pv.tile([128, JBLK], F32, tag='pv')
                for kk in range(SK):
                    kks = slice(kk * 128, (kk + 1) * 128)
                    psl = ps_qk.tile([128, 2 * JBLK], F32, tag='qk')
                    nc.tensor.matmul(psl[:, 0:JBLK],
                                     KTs[p][0:64, kks], QT[p][0:64, js],
                                     start=True, stop=True, tile_position=(0, 0))
                    nc.tensor.matmul(psl[:, JBLK:2 * JBLK],
                                     KTs[p][64:128, kks], QT[p][64:128, js],
                                     start=True, stop=True, tile_position=(64, 0))
                    ex = expool.tile([128, 2 * JBLK], BF16, tag='ex')
                    nc.scalar.activation(ex, psl, mybir.ActivationFunctionType.Exp)
                    if _DEBUG and J == 0 and p == 0 and kk == 0:
                        de = outpool.tile([128, 2 * JBLK], F32, tag='de')
                        nc.vector.tensor_copy(de, ex)
                        nc.sync.dma_start(out=dbg['d_ex'].ap(), in_=de)
                    for hh, pv in enumerate((pv0, pv1)):
                        hcol = 2 * p + hh
                        nc.tensor.matmul(
                            pv[0:DH + 1, :],
                            v_sb[kk][:, hcol, :],
                            ex[:, hh * JBLK:(hh + 1) * JBLK],
                            start=(kk == 0), stop=(kk == SK - 1))
                # normalize: ctxT[d, q] * (1/denom[q]) via partition broadcast
                ct = ctxpool.tile([128, JBLK], BF16, tag=f'ctx{p}')
                for hh, pv in enumerate((pv0, pv1)):
                    # single eviction frees the PSUM slot; normalize from SBUF
                    rawct = small.tile([128, JBLK], F32, tag='rawct')
                    nc.vector.tensor_copy(rawct[0:DH + 1, :], pv[0:DH + 1, :])
                    if _DEBUG and J == 0 and p == 0 and hh == 0:
                        dr = outpool.tile([128, JBLK], F32, tag='dr')
                        nc.vector.tensor_copy(dr[0:DH + 1, :], rawct[0:DH + 1, :])
                        nc.sync.dma_start(out=dbg['d_raw'].ap(), in_=dr)
                    rec = small.tile([128, JBLK], F32, tag='rec')
                    nc.vector.reciprocal_approx_fast(rec[0:DH + 1, :], rawct[0:DH + 1, :])
                    bcs = small.tile([128, JBLK], F32, tag='bcs')
                    nc.sync.dma_start(out=bcs[0:1, :], in_=rec[DH:DH + 1, :])
                    bc = small.tile([128, JBLK], F32, tag='bc')
                    nc.gpsimd.partition_broadcast(bc[0:DH, :], bcs[0:1, :])
                    if hh == 0:
                        nc.vector.tensor_mul(ct[0:DH, :], rawct[0:DH, :], bc[0:DH, :])
                    else:
                        tmp = small.tile([128, JBLK], BF16, tag='tmp')
                        nc.vector.tensor_mul(tmp[0:DH, :], rawct[0:DH, :], bc[0:DH, :])
                        nc.sync.dma_start(out=ct[DH:128, :], in_=tmp[0:DH, :])
                if _DEBUG and J == 0 and p == 0:
                    dc = outpool.tile([128, JBLK], F32, tag='dc')
                    nc.vector.tensor_copy(dc, ct)
                    nc.sync.dma_start(out=dbg['d_ctx'].ap(), in_=dc)
                ctx_tiles.append(ct)
                if p == 0 and pend_out is not None:
                    emit_out(*pend_out)
                    pend_out = None
            pend_out = (J, ctx_tiles)
        emit_out(*pend_out)

    nc.compile()
    return nc


def _get_nc():
    if 'nc' not in _CACHE:
        _CACHE['nc'] = _build()
    return _CACHE['nc']


def shard_inputs(x, y, bias, wq, wk, wv, wo):
    """Build the 8 per-core input maps from full inputs."""
    scale = (H // NH) ** -0.5
    wqs = (wq * scale).astype(np.float32)
    in_maps = []
    for c in range(NCORES):
        b = c // (NCORES // B)
        g = c % (NCORES // B)
        cols = slice(g * C, (g + 1) * C)
        eb = np.exp(bias[b, 0, 0, :].astype(np.float64)).astype(np.float32)
        bf = ml_dtypes.bfloat16
        in_maps.append({
            'xT': np.ascontiguousarray(x[b].T.astype(bf)),
            'yT': np.ascontiguousarray(y[b].T.astype(bf)),
            'wq': np.ascontiguousarray(wqs[:, cols].astype(bf)),
            'wk': np.ascontiguousarray(wk[:, cols].astype(bf)),
            'wv': np.ascontiguousarray(wv[:, cols].astype(bf)),
            'wo': np.ascontiguousarray(wo[cols, :].astype(bf)),
            'ebias': np.ascontiguousarray(eb.reshape(SK, 128).T),
        })
    return in_maps


def kernel(x, y, bias, wq, wk, wv, wo, _trace=False):
    x, y, bias = np.asarray(x), np.asarray(y), np.asarray(bias)
    wq, wk, wv, wo = (np.asarray(t) for t in (wq, wk, wv, wo))
    nc = _get_nc()
    in_maps = shard_inputs(x, y, bias, wq, wk, wv, wo)
    kw = {}
    if _trace:
        kw = dict(trace=True, stitch_traces=False)
    res = bass_utils.run_bass_kernel_spmd(nc, in_maps, core_ids=list(range(NCORES)), **kw)
    full = np.zeros((B, S, H), dtype=np.float64)
    for c in range(NCORES):
        full[c // (NCORES // B)] += res.results[c]['out'].astype(np.float64)
    if _trace:
        _CACHE['last_results'] = res
    return full.astype(np.float32)


# revision 14
# speedup vs baseline: 1.4866x; 1.0133x over previous
"""Multi-head attention (B=2, S=2048, H=1024, 16 heads) on 8 TRN2 NeuronCores.

Sharding: data parallel on batch (2) x tensor parallel on heads (4 heads/core,
Megatron column-split qkv, row-split wo). Host pre-transposes x/y, pre-scales
wq by dh^-0.5, and sum-reduces the 4 partial outputs per batch element.

Per-core kernel:
  Projections (bf16): QT/KT in [head-pair-dims(128) x S] transposed layout,
  V in natural [S x dims] layout with a fused ones column scaled by exp(bias)
  (exact additive-bias support). The K/Q/V projection work is emitted as
  small PSUM groups: a minimal prefix runs before attention starts, the rest
  is woven into the attention k-loop to fill TensorE idle slots while the
  Scalar engine (exp) paces the pipeline.

  Attention per 512-wide q-block and head pair: row-tiled (2-head packed)
  QK^T -> logitsT psum [128,1024] -> one ACT exp per pair (psum->sbuf bf16)
  -> PV matmul with fused denominator row (fp32 accumulate) -> fast
  reciprocal + gpsimd partition_broadcast normalize -> pair-stacked bf16
  output projection, deferred one block for overlap.
"""
import sys
sys.path.insert(0, '/opt/trn_rl_repo')
from collections import deque
from contextlib import ExitStack

import numpy as np
import ml_dtypes

import concourse.bacc as bacc
import concourse.tile as tile
from concourse import mybir
from concourse import bass_utils

B, S, H, NH = 2, 2048, 1024, 16
DH = H // NH            # 64
NCORES = 8
HPC = NH // (NCORES // B)   # 4 heads per core
C = HPC * DH            # 256 projected cols per core
KT_H = H // 128         # 8 contraction tiles over H
SK = S // 128           # 16 s-subtiles
JBLK = 512
NJ = S // JBLK          # 4 q-blocks
F32 = mybir.dt.float32
F32R = mybir.dt.float32r
BF16 = mybir.dt.bfloat16

_CACHE = {}
_DEBUG = False


def _build():
    nc = bacc.Bacc('TRN2', debug=False, num_devices=NCORES)
    xT = nc.dram_tensor('xT', [H, S], BF16, kind='ExternalInput')
    yT = nc.dram_tensor('yT', [H, S], BF16, kind='ExternalInput')
    wq = nc.dram_tensor('wq', [H, C], BF16, kind='ExternalInput')
    wk = nc.dram_tensor('wk', [H, C], BF16, kind='ExternalInput')
    wv = nc.dram_tensor('wv', [H, C], BF16, kind='ExternalInput')
    wo = nc.dram_tensor('wo', [C, H], BF16, kind='ExternalInput')
    ebias = nc.dram_tensor('ebias', [128, SK], F32, kind='ExternalInput')
    out = nc.dram_tensor('out', [S, H], F32, kind='ExternalOutput')

    with tile.TileContext(nc) as tc, ExitStack() as ctx:
        res = ctx.enter_context(tc.tile_pool(name='res', bufs=1))
        expool = ctx.enter_context(tc.tile_pool(name='expool', bufs=3))
        ctxpool = ctx.enter_context(tc.tile_pool(name='ctxpool', bufs=2))
        small = ctx.enter_context(tc.tile_pool(name='small', bufs=2))
        outpool = ctx.enter_context(tc.tile_pool(name='outpool', bufs=3))
        ps_qk = ctx.enter_context(tc.tile_pool(name='ps_qk', bufs=2, space='PSUM'))
        ps_pv = ctx.enter_context(tc.tile_pool(name='ps_pv', bufs=4, space='PSUM'))

        # ---- input DMAs, ordered for earliest compute start ----
        wq_r = res.tile([128, KT_H, C], BF16, tag='wq')
        wk_r = res.tile([128, KT_H, C], BF16, tag='wk')
        wv_r = res.tile([128, KT_H, C], BF16, tag='wv')
        nc.sync.dma_start(out=wk_r, in_=wk.ap().rearrange('(t p) c -> p t c', p=128))
        nc.sync.dma_start(out=wv_r, in_=wv.ap().rearrange('(t p) c -> p t c', p=128))
        nc.sync.dma_start(out=wq_r, in_=wq.ap().rearrange('(t p) c -> p t c', p=128))
        eb = res.tile([128, SK], F32, tag='eb')
        nc.sync.dma_start(out=eb, in_=ebias.ap())
        ones4 = res.tile([128, HPC, 1], F32, tag='ones4')
        nc.vector.memset(ones4, 1.0)

        xT_ap, yT_ap = xT.ap(), yT.ap()
        HB = S // 2
        xts = [[res.tile([128, HB], BF16, tag=f'xts{k}_{j}', name=f'xts{k}_{j}')
                for j in range(2)] for k in range(KT_H)]
        yts = [[res.tile([128, HB], BF16, tag=f'yts{k}_{j}', name=f'yts{k}_{j}')
                for j in range(2)] for k in range(KT_H)]
        for j in range(2):
            hs = slice(j * HB, (j + 1) * HB)
            for k in range(KT_H):
                ks = slice(k * 128, (k + 1) * 128)
                nc.scalar.dma_start(out=yts[k][j], in_=yT_ap[ks, hs])
                nc.scalar.dma_start(out=xts[k][j], in_=xT_ap[ks, hs])
        wo_r = res.tile([128, 2, H], BF16, tag='wo')
        nc.sync.dma_start(out=wo_r, in_=wo.ap().rearrange('(t p) n -> p t n', p=128))

        # ---- resident activations ----
        QT = [res.tile([128, S], BF16, tag=f'qt{p}', name=f'qt{p}') for p in range(2)]
        KTs = [res.tile([128, S], BF16, tag=f'kt{p}', name=f'kt{p}') for p in range(2)]
        v_sb = [res.tile([128, HPC, DH + 1], BF16, tag=f'v{i}', name=f'v{i}')
                for i in range(SK)]

        # ---- projection groups (8 matmuls + eviction), run direct or woven ----
        gid = [0]

        def qk_group(which, p, j4):
            w_r = wq_r if which == 'q' else wk_r
            src = xts if which == 'q' else yts
            dest = QT[p] if which == 'q' else KTs[p]
            js = slice(j4 * JBLK, (j4 + 1) * JBLK)
            hj = slice((j4 % 2) * JBLK, (j4 % 2 + 1) * JBLK)
            cs = slice(p * 128, (p + 1) * 128)
            gid[0] += 1
            ps = ps_pv.tile([128, JBLK], F32, tag='pv', name=f'g{gid[0]}')
            items = []
            for k in range(KT_H):
                def mm(k=k):
                    nc.tensor.matmul(ps, w_r[:, k, cs], src[k][j4 // 2][:, hj],
                                     start=(k == 0), stop=(k == KT_H - 1))
                items.append(mm)

            def fin():
                nc.vector.tensor_copy(dest[:, js], ps)
            items.append(fin)
            return items

        def v_group(j4, m):
            sub = j4 * 4 + m
            hj0 = (j4 % 2) * JBLK + m * 128
            gid[0] += 1
            ps = ps_pv.tile([128, JBLK], F32, tag='pv', name=f'g{gid[0]}')
            items = []
            for k in range(KT_H):
                def mm(k=k):
                    nc.tensor.matmul(ps[:, 0:C],
                                     yts[k][j4 // 2][:, hj0:hj0 + 128],
                                     wv_r[:, k, :],
                                     start=(k == 0), stop=(k == KT_H - 1))
                items.append(mm)

            def fin():
                nc.vector.tensor_scalar_mul(
                    v_sb[sub][:, :, 0:DH],
                    ps[:, 0:C].rearrange('p (h c) -> p h c', h=HPC),
                    eb[:, sub:sub + 1])
                nc.gpsimd.tensor_scalar_mul(v_sb[sub][:, :, DH:DH + 1], ones4,
                                            eb[:, sub:sub + 1])
            items.append(fin)
            return items

        # prefix: everything attention block (J0,p0) touches
        for grp in ([qk_group('k', 0, j4) for j4 in range(NJ)]
                    + [qk_group('q', 0, 0)]
                    + [v_group(j4, m) for j4 in range(NJ) for m in range(4)]):
            for it in grp:
                it()

        # woven into the attention k-loop (2 items/step meets all deadlines)
        weave = deque()
        for grp in ([qk_group('k', 1, 0)]
                    + [qk_group('q', 1, 0)]
                    + [qk_group('k', 1, j4) for j4 in range(1, NJ)]
                    + [qk_group('q', 0, 1), qk_group('q', 1, 1),
                       qk_group('q', 0, 2), qk_group('q', 1, 2),
                       qk_group('q', 0, 3), qk_group('q', 1, 3)]):
            weave.extend(grp)

        def weave_emit(n):
            for _ in range(n):
                if weave:
                    weave.popleft()()

        # ---- attention + output projection ----
        dbg = {}
        if _DEBUG:
            for nm, shp in [('d_ex', [128, 2 * JBLK]), ('d_raw', [128, JBLK]),
                            ('d_ctx', [128, JBLK])]:
                dbg[nm] = nc.dram_tensor(nm, shp, F32, kind='ExternalOutput')

        pend_out = None

        def emit_out(J, ctx_tiles):
            for m in range(4):
                ms = slice(m * 128, (m + 1) * 128)
                for n in range(2):
                    ns = slice(n * JBLK, (n + 1) * JBLK)
                    pso = ps_pv.tile([128, JBLK], F32, tag='pv')
                    for p in range(2):
                        nc.tensor.matmul(pso, ctx_tiles[p][:, ms], wo_r[:, p, ns],
                                         start=(p == 0), stop=(p == 1))
                    ob = outpool.tile([128, JBLK], F32, tag='ob')
                    nc.vector.tensor_copy(ob, pso)
                    nc.sync.dma_start(out=out.ap()[J * JBLK + m * 128:
                                                   J * JBLK + (m + 1) * 128, ns],
                                      in_=ob)

        for J in range(NJ):
            js = slice(J * JBLK, (J + 1) * JBLK)
            ctx_tiles = []
            for p in range(2):
                pv0 = ps_pv.tile([128, JBLK], F32, tag='pv')
                pv1 = ps_pv.tile([128, JBLK], F32, tag='pv')
                for kk in range(SK):
                    kks = slice(kk * 128, (kk + 1) * 128)
                    psl = ps_qk.tile([128, 2 * JBLK], F32, tag='qk')
                    nc.tensor.matmul(psl[:, 0:JBLK],
                                     KTs[p][0:64, kks], QT[p][0:64, js],
                                     start=True, stop=True, tile_position=(0, 0))
                    nc.tensor.matmul(psl[:, JBLK:2 * JBLK],
                                     KTs[p][64:128, kks], QT[p][64:128, js],
                                     start=True, stop=True, tile_position=(64, 0))
                    weave_emit(2)
                    ex = expool.tile([128, 2 * JBLK], BF16, tag='ex')
                    nc.scalar.activation(ex, psl, mybir.ActivationFunctionType.Exp)
                    if _DEBUG and J == 0 and p == 0 and kk == 0:
                        de = outpool.tile([128, 2 * JBLK], F32, tag='de')
                        nc.vector.tensor_copy(de, ex)
                        nc.sync.dma_start(out=dbg['d_ex'].ap(), in_=de)
                    for hh, pv in enumerate((pv0, pv1)):
                        hcol = 2 * p + hh
                        nc.tensor.matmul(
                            pv[0:DH + 1, :],
                            v_sb[kk][:, hcol, :],
                            ex[:, hh * JBLK:(hh + 1) * JBLK],
                            start=(kk == 0), stop=(kk == SK - 1))
                # normalize: ctxT[d, q] * (1/denom[q]) via partition broadcast
                ct = ctxpool.tile([128, JBLK], BF16, tag=f'ctx{p}')
                for hh, pv in enumerate((pv0, pv1)):
                    rawct = small.tile([128, JBLK], F32, tag='rawct')
                    nc.vector.tensor_copy(rawct[0:DH + 1, :], pv[0:DH + 1, :])
                    if _DEBUG and J == 0 and p == 0 and hh == 0:
                        dr = outpool.tile([128, JBLK], F32, tag='dr')
                        nc.vector.tensor_copy(dr[0:DH + 1, :], rawct[0:DH + 1, :])
                        nc.sync.dma_start(out=dbg['d_raw'].ap(), in_=dr)
                    rec = small.tile([128, JBLK], F32, tag='rec')
                    nc.vector.reciprocal_approx_fast(rec[0:DH + 1, :],
                                                     rawct[0:DH + 1, :])
                    bcs = small.tile([128, JBLK], F32, tag='bcs')
                    nc.sync.dma_start(out=bcs[0:1, :], in_=rec[DH:DH + 1, :])
                    bc = small.tile([128, JBLK], F32, tag='bc')
                    nc.gpsimd.partition_broadcast(bc[0:DH, :], bcs[0:1, :])
                    if hh == 0:
                        nc.vector.tensor_mul(ct[0:DH, :], rawct[0:DH, :], bc[0:DH, :])
                    else:
                        tmp = small.tile([128, JBLK], BF16, tag='tmp')
                        nc.vector.tensor_mul(tmp[0:DH, :], rawct[0:DH, :], bc[0:DH, :])
                        nc.sync.dma_start(out=ct[DH:128, :], in_=tmp[0:DH, :])
                if _DEBUG and J == 0 and p == 0:
                    dc = outpool.tile([128, JBLK], F32, tag='dc')
                    nc.vector.tensor_copy(dc, ct)
                    nc.sync.dma_start(out=dbg['d_ctx'].ap(), in_=dc)
                ctx_tiles.append(ct)
                if p == 0 and pend_out is not None:
                    emit_out(*pend_out)
                    pend_out = None
            pend_out = (J, ctx_tiles)
        weave_emit(len(weave))
        emit_out(*pend_out)

    nc.compile()
    return nc


def _get_nc():
    if 'nc' not in _CACHE:
        _CACHE['nc'] = _build()
    return _CACHE['nc']


def shard_inputs(x, y, bias, wq, wk, wv, wo):
    """Build the 8 per-core input maps from full inputs."""
    scale = (H // NH) ** -0.5
    wqs = (wq * scale).astype(np.float32)
    bf = ml_dtypes.bfloat16
    in_maps = []
    for c in range(NCORES):
        b = c // (NCORES // B)
        g = c % (NCORES // B)
        cols = slice(g * C, (g + 1) * C)
        eb = np.exp(bias[b, 0, 0, :].astype(np.float64)).astype(np.float32)
        in_maps.append({
            'xT': np.ascontiguousarray(x[b].T.astype(bf)),
            'yT': np.ascontiguousarray(y[b].T.astype(bf)),
            'wq': np.ascontiguousarray(wqs[:, cols].astype(bf)),
            'wk': np.ascontiguousarray(wk[:, cols].astype(bf)),
            'wv': np.ascontiguousarray(wv[:, cols].astype(bf)),
            'wo': np.ascontiguousarray(wo[cols, :].astype(bf)),
            'ebias': np.ascontiguousarray(eb.reshape(SK, 128).T),
        })
    return in_maps


def kernel(x, y, bias, wq, wk, wv, wo, _trace=False):
    x, y, bias = np.asarray(x), np.asarray(y), np.asarray(bias)
    wq, wk, wv, wo = (np.asarray(t) for t in (wq, wk, wv, wo))
    nc = _get_nc()
    in_maps = shard_inputs(x, y, bias, wq, wk, wv, wo)
    kw = {}
    if _trace:
        kw = dict(trace=True, stitch_traces=False)
    res = bass_utils.run_bass_kernel_spmd(nc, in_maps, core_ids=list(range(NCORES)), **kw)
    full = np.zeros((B, S, H), dtype=np.float64)
    for c in range(NCORES):
        full[c // (NCORES // B)] += res.results[c]['out'].astype(np.float64)
    if _trace:
        _CACHE['last_results'] = res
    return full.astype(np.float32)
